# revision 5
# baseline (speedup 1.0000x reference)
"""Distributed Trainium2 Bass kernel: masked (upper-triangular) attention.

reference (L=4096, D=1024, fp32):
    Q = x @ Wq + bq ; K = z @ Wk + bk ; V = z @ Wv + bv
    S = Q @ K.T ; S[row > col] = -inf
    out = softmax(S / sqrt(D)) @ V

Strategy (8 NeuronCores, one TRN2 chip, SPMD):
  - Sequence parallel on query rows: core c owns rows [512c, 512c+512).
  - K/V projection sharded over z rows (512/core), AllGathered through a
    chip-shared DRAM buffer (K stored transposed [D, L], V natural [L, D]).
  - Attention computed as S^T tiles (keys on partitions) so the P^T needed by
    the PV matmul comes straight out of the softmax with no transposes.
  - Softmax without max-subtraction (scores here are O(1), exp can't
    overflow in fp32); mask applied multiplicatively after exp, built at
    runtime from an iota constant + a per-core row0 scalar input, keeping
    one graph valid for all cores.
  - All matmuls in fp32r (TF32, ~1e-4 rel err) with fp32 PSUM accumulation.
"""

import math

import numpy as np

import concourse.bass as bass  # noqa: F401  (kept for API parity/debugging)
import concourse.mybir as mybir
import concourse.tile as tile
from concourse import bacc
from concourse.bass_utils import run_bass_kernel_spmd

F32 = mybir.dt.float32
F32R = mybir.dt.float32r
AF = mybir.ActivationFunctionType
OP = mybir.AluOpType
P = 128
NCORES = 8

L = 4096
D = 1024


def build_graph(Ldim=L, Ddim=D):
    nc = bacc.Bacc("TRN2", target_bir_lowering=False, debug=False, num_devices=NCORES)
    ROWS = Ldim // NCORES        # query rows per core
    MB = ROWS // P               # 128-row m-chunks per core (4)
    ZB = ROWS // P               # z-shard 128-row blocks (z shard == ROWS rows)
    SW = ROWS                    # key-tile width == z-shard width
    JT = SW // P                 # 128-row subtiles per key tile
    NT = NCORES                  # one key tile per shard
    IO = Ddim // P               # contraction chunks (8)
    AO = Ddim // P               # d_attn 128-blocks (8)
    VH = Ddim // 512             # 512-wide value column halves (2)
    HLF = ROWS // 256            # 256-row halves for PV psum pressure (2)
    scale = 1.0 / math.sqrt(Ddim)

    x_ext = nc.declare_dram_parameter("x", [ROWS, Ddim], F32, isOutput=False)
    z_ext = nc.declare_dram_parameter("z", [ROWS, Ddim], F32, isOutput=False)
    wq_ext = nc.declare_dram_parameter("Wq", [Ddim, Ddim], F32, isOutput=False)
    wk_ext = nc.declare_dram_parameter("Wk", [Ddim, Ddim], F32, isOutput=False)
    wv_ext = nc.declare_dram_parameter("Wv", [Ddim, Ddim], F32, isOutput=False)
    bq_ext = nc.declare_dram_parameter("bq", [Ddim], F32, isOutput=False)
    bk_ext = nc.declare_dram_parameter("bk", [Ddim], F32, isOutput=False)
    bv_ext = nc.declare_dram_parameter("bv", [Ddim], F32, isOutput=False)
    row0_ext = nc.declare_dram_parameter("row0", [1], F32, isOutput=False)
    out_ext = nc.declare_dram_parameter("out", [ROWS, Ddim], F32, isOutput=True)

    ident_d = nc.inline_tensor(np.eye(P, dtype=np.float32), name="ident_c")
    ones_d = nc.inline_tensor(np.ones((P, 8), np.float32), name="ones_c")
    # iota[p, j, m] = m - p - 128j ; mask keeps where iota + row0 - SW*t <= 0
    iota_np = (np.arange(ROWS)[None, None, :] - np.arange(P)[:, None, None]
               - 128 * np.arange(JT)[None, :, None]).astype(np.float32)
    iota_d = nc.inline_tensor(np.ascontiguousarray(iota_np), name="iota_c")
    # n512[p, t] = -SW*t
    n512_d = nc.inline_tensor(
        np.broadcast_to((-float(SW) * np.arange(NT, dtype=np.float32))[None, :], (P, NT)).copy(),
        name="n512_c")

    with tile.TileContext(nc) as tc:
        with tc.tile_pool(name="const", bufs=1) as constp, \
             tc.tile_pool(name="persist", bufs=1) as persist, \
             tc.tile_pool(name="dram", bufs=1, space="DRAM") as dram:
            ident = constp.tile([P, P], F32R)
            nc.gpsimd.dma_start(out=ident[:], in_=ident_d.ap())
            ones8 = constp.tile([P, 8], F32R)
            nc.gpsimd.dma_start(out=ones8[:], in_=ones_d.ap())
            iota = constp.tile([P, JT, ROWS], F32)
            nc.sync.dma_start(out=iota[:], in_=iota_d.ap())
            bvb = constp.tile([P, Ddim], F32)
            nc.gpsimd.dma_start(out=bvb[:], in_=bv_ext[:].partition_broadcast(P))
            bqs = constp.tile([P, AO], F32)
            nc.sync.dma_start(out=bqs[:], in_=bq_ext[:].rearrange("(ao p) -> p ao", p=P))
            bks = constp.tile([P, AO], F32)
            nc.sync.dma_start(out=bks[:], in_=bk_ext[:].rearrange("(ao p) -> p ao", p=P))
            row0b = constp.tile([P, 1], F32)
            nc.sync.dma_start(out=row0b[:], in_=row0_ext[:].partition_broadcast(P))
            # r0t[p, t] = row0 - 512t
            n512 = constp.tile([P, NT], F32)
            nc.sync.dma_start(out=n512[:], in_=n512_d.ap())
            r0t = constp.tile([P, NT], F32)
            nc.vector.tensor_scalar(r0t[:], n512[:], row0b[:], None, OP.add)

            QT = persist.tile([P, IO, ROWS], F32R)
            kt_b = dram.tile([Ddim, ROWS], F32R)
            v_b = dram.tile([ROWS, Ddim], F32R)
            kt_g = dram.tile([NCORES, Ddim, ROWS], F32R, addr_space="Shared")
            v_g = dram.tile([NCORES, ROWS, Ddim], F32R, addr_space="Shared")

            # ---------------- Phase 1: K/V projection of own z shard ----------------
            with tc.tile_pool(name="wkv", bufs=1) as wp, \
                 tc.tile_pool(name="zp", bufs=1) as zp, \
                 tc.tile_pool(name="kvs", bufs=1) as kvsp, \
                 tc.tile_pool(name="tpp", bufs=2, space="PSUM") as tpp, \
                 tc.tile_pool(name="pp", bufs=2, space="PSUM") as pp:
                wk = wp.tile([P, IO, Ddim], F32R)
                nc.gpsimd.dma_start(out=wk[:], in_=wk_ext[:].rearrange("(io p) a -> p io a", p=P))
                wv = wp.tile([P, IO, Ddim], F32R)
                nc.gpsimd.dma_start(out=wv[:], in_=wv_ext[:].rearrange("(io p) a -> p io a", p=P))
                zsb = zp.tile([P, ZB, Ddim], F32R)
                nc.gpsimd.dma_start(out=zsb[:], in_=z_ext[:].rearrange("(nb p) i -> p nb i", p=P))
                zT = zp.tile([P, IO, ROWS], F32R)
                for nb in range(ZB):
                    for io in range(IO):
                        tp = tpp.tile([P, P], F32R, tag="tp")
                        nc.tensor.transpose(tp[:], zsb[:, nb, io * P:(io + 1) * P], ident[:])
                        nc.scalar.activation(zT[:, io, nb * P:(nb + 1) * P], tp[:], AF.Copy)

                KTs = kvsp.tile([P, AO, ROWS], F32R)
                for ao in range(AO):
                    kp = pp.tile([P, ROWS], F32, tag="kp")
                    for io in range(IO):
                        nc.tensor.matmul(kp[:], wk[:, io, ao * P:(ao + 1) * P], zT[:, io, :],
                                         start=(io == 0), stop=(io == IO - 1))
                    nc.vector.tensor_scalar(KTs[:, ao, :], kp[:], bks[:, ao:ao + 1], None, OP.add)

                Vs = kvsp.tile([P, ZB, Ddim], F32R)
                for nb in range(ZB):
                    vp = pp.tile([P, Ddim], F32, tag="vp")
                    for io in range(IO):
                        for vh in range(VH):
                            nc.tensor.matmul(vp[:, vh * 512:(vh + 1) * 512],
                                             zT[:, io, nb * P:(nb + 1) * P],
                                             wv[:, io, vh * 512:(vh + 1) * 512],
                                             start=(io == 0), stop=(io == IO - 1))
                    nc.vector.tensor_tensor(Vs[:, nb, :], vp[:], bvb[:], OP.add)

                nc.sync.dma_start(out=kt_b[:].rearrange("(ao p) n -> p ao n", p=P), in_=KTs[:])
                nc.sync.dma_start(out=v_b[:].rearrange("(nb p) v -> p nb v", p=P), in_=Vs[:])

            nc.gpsimd.collective_compute(
                "AllGather", OP.bypass, replica_groups=[list(range(NCORES))],
                ins=[kt_b[:].opt()], outs=[kt_g[:].opt()])
            nc.gpsimd.collective_compute(
                "AllGather", OP.bypass, replica_groups=[list(range(NCORES))],
                ins=[v_b[:].opt()], outs=[v_g[:].opt()])

            # ---------------- Phase 2: Q^T projection (overlaps the AllGather) ----------------
            with tc.tile_pool(name="wqp", bufs=1) as wqp, \
                 tc.tile_pool(name="xp", bufs=1) as xp, \
                 tc.tile_pool(name="tpq", bufs=2, space="PSUM") as tpq, \
                 tc.tile_pool(name="qpp", bufs=2, space="PSUM") as qpp:
                wq = wqp.tile([P, IO, Ddim], F32R)
                nc.gpsimd.dma_start(out=wq[:], in_=wq_ext[:].rearrange("(io p) a -> p io a", p=P))
                xsb = xp.tile([P, MB, Ddim], F32R)
                nc.gpsimd.dma_start(out=xsb[:], in_=x_ext[:].rearrange("(mb p) i -> p mb i", p=P))
                xT = xp.tile([P, IO, ROWS], F32R)
                for mb in range(MB):
                    for io in range(IO):
                        tq = tpq.tile([P, P], F32R, tag="tq")
                        nc.tensor.transpose(tq[:], xsb[:, mb, io * P:(io + 1) * P], ident[:])
                        nc.scalar.activation(xT[:, io, mb * P:(mb + 1) * P], tq[:], AF.Copy)
                for ao in range(AO):
                    qp = qpp.tile([P, ROWS], F32, tag="qp")
                    for io in range(IO):
                        nc.tensor.matmul(qp[:], wq[:, io, ao * P:(ao + 1) * P], xT[:, io, :],
                                         start=(io == 0), stop=(io == IO - 1))
                    # fold the softmax 1/sqrt(D) into Q^T
                    nc.vector.tensor_scalar(QT[:, ao, :], qp[:], bqs[:, ao:ao + 1], float(scale),
                                            OP.add, OP.mult)

            # ---------------- Phase 3: attention over gathered K^T / V ----------------
            acc = persist.tile([P, MB, Ddim], F32)       # PV accumulator (SBUF)
            lacc = persist.tile([P, MB, 8], F32)         # row-sum accumulator
            with tc.tile_pool(name="kvt", bufs=2) as kvt, \
                 tc.tile_pool(name="esp", bufs=2) as esp, \
                 tc.tile_pool(name="mkp", bufs=2) as mkp, \
                 tc.tile_pool(name="spp", bufs=2, space="PSUM") as spp, \
                 tc.tile_pool(name="pvp", bufs=1, space="PSUM") as pvp, \
                 tc.tile_pool(name="lpp", bufs=1, space="PSUM") as lpp:
                for t in range(NT):
                    ktt = kvt.tile([P, IO, SW], F32R, tag="ktt")
                    nc.sync.dma_start(out=ktt[:], in_=kt_g[t].rearrange("(ao p) n -> p ao n", p=P))
                    vtt = kvt.tile([P, JT, Ddim], F32R, tag="vtt")
                    nc.sync.dma_start(out=vtt[:], in_=v_g[t].rearrange("(nb p) v -> p nb v", p=P))

                    es = esp.tile([P, JT, ROWS], F32R, tag="es")
                    for j in range(JT):
                        sp = spp.tile([P, ROWS], F32, tag="sp")
                        for io in range(IO):
                            nc.tensor.matmul(sp[:], ktt[:, io, j * P:(j + 1) * P],
                                             QT[:, io, :], start=(io == 0), stop=(io == IO - 1))
                        nc.scalar.activation(es[:, j, :], sp[:], AF.Exp)
                    # multiplicative causal mask: keep where iota + (row0 - 512t) <= 0
                    mk = mkp.tile([P, JT, ROWS], F32, tag="mk")
                    nc.vector.tensor_scalar(mk[:], iota[:], r0t[:, t:t + 1], 0.0,
                                            OP.add, OP.is_le)
                    nc.vector.tensor_tensor(es[:], es[:], mk[:], OP.mult)

                    for h in range(HLF):
                        pvs = [pvp.tile([P, Ddim], F32, tag=f"pv{mc}", name=f"pv{mc}_{t}_{h}") for mc in range(2)]
                        lps = [lpp.tile([P, 8], F32, tag=f"lp{mc}", name=f"lp{mc}_{t}_{h}") for mc in range(2)]
                        for j in range(JT):
                            for mc in range(2):
                                m0 = h * 256 + mc * P
                                lhs = es[:, j, m0:m0 + P]
                                nc.tensor.matmul(lps[mc][:], lhs, ones8[:],
                                                 start=(j == 0), stop=(j == JT - 1))
                                for vh in range(VH):
                                    nc.tensor.matmul(pvs[mc][:, vh * 512:(vh + 1) * 512],
                                                     lhs, vtt[:, j, vh * 512:(vh + 1) * 512],
                                                     start=(j == 0), stop=(j == JT - 1))
                        for mc in range(2):
                            gmc = 2 * h + mc
                            if t == 0:
                                nc.vector.tensor_copy(acc[:, gmc, :], pvs[mc][:])
                                nc.vector.tensor_copy(lacc[:, gmc, :], lps[mc][:])
                            else:
                                nc.vector.tensor_tensor(acc[:, gmc, :], acc[:, gmc, :],
                                                        pvs[mc][:], OP.add)
                                nc.vector.tensor_tensor(lacc[:, gmc, :], lacc[:, gmc, :],
                                                        lps[mc][:], OP.add)

                # normalize and write out
                with tc.tile_pool(name="recp", bufs=1) as recp:
                    for gmc in range(MB):
                        rec = recp.tile([P, 1], F32, tag="rec")
                        nc.vector.reciprocal(rec[:], lacc[:, gmc, 0:1])
                        nc.vector.tensor_scalar(acc[:, gmc, :], acc[:, gmc, :], rec[:],
                                                None, OP.mult)
                    nc.sync.dma_start(out=out_ext[:].rearrange("(mb p) v -> p mb v", p=P),
                                      in_=acc[:])
    nc.compile()
    return nc


_GRAPH_CACHE = {}


def _get_graph(Ldim=L, Ddim=D):
    key = (Ldim, Ddim)
    if key not in _GRAPH_CACHE:
        _GRAPH_CACHE[key] = build_graph(Ldim, Ddim)
    return _GRAPH_CACHE[key]


def kernel(x, z, Wq, bq, Wk, bk, Wv, bv):
    x = np.ascontiguousarray(np.asarray(x, dtype=np.float32))
    z = np.ascontiguousarray(np.asarray(z, dtype=np.float32))
    Ldim, Ddim = x.shape
    nc = _get_graph(Ldim, Ddim)
    ROWS = Ldim // NCORES
    common = {
        "Wq": np.asarray(Wq, np.float32), "bq": np.asarray(bq, np.float32),
        "Wk": np.asarray(Wk, np.float32), "bk": np.asarray(bk, np.float32),
        "Wv": np.asarray(Wv, np.float32), "bv": np.asarray(bv, np.float32),
    }
    in_maps = []
    for c in range(NCORES):
        m = dict(common)
        m["x"] = x[ROWS * c:ROWS * (c + 1)]
        m["z"] = z[ROWS * c:ROWS * (c + 1)]
        m["row0"] = np.array([ROWS * c], dtype=np.float32)
        in_maps.append(m)
    res = run_bass_kernel_spmd(nc, in_maps, core_ids=list(range(NCORES)))
    out = np.empty((Ldim, Ddim), dtype=np.float32)
    for c in range(NCORES):
        out[ROWS * c:ROWS * (c + 1)] = res.results[c]["out"]
    return out


# revision 6
# speedup vs baseline: 1.0189x; 1.0189x over previous
"""Distributed Trainium2 Bass kernel: masked (upper-triangular) attention.

reference (L=4096, D=1024, fp32):
    Q = x @ Wq + bq ; K = z @ Wk + bk ; V = z @ Wv + bv
    S = Q @ K.T ; S[row > col] = -inf
    out = softmax(S / sqrt(D)) @ V

Strategy (8 NeuronCores, one TRN2 chip, SPMD):
  - Sequence parallel on query rows: core c owns rows [512c, 512c+512).
  - K/V projection sharded over z rows (512/core), AllGathered through a
    chip-shared DRAM buffer (K stored transposed [D, L], V natural [L, D]).
  - Attention computed as S^T tiles (keys on partitions) so the P^T needed by
    the PV matmul comes straight out of the softmax with no transposes.
  - Softmax without max-subtraction (scores here are O(1), exp can't
    overflow in fp32); mask applied multiplicatively after exp, built at
    runtime from an iota constant + a per-core row0 scalar input, keeping
    one graph valid for all cores.
  - All matmuls in fp32r (TF32, ~1e-4 rel err) with fp32 PSUM accumulation.
"""

import math

import numpy as np

import concourse.bass as bass  # noqa: F401  (kept for API parity/debugging)
import concourse.mybir as mybir
import concourse.tile as tile
from concourse import bacc
from concourse.bass_utils import run_bass_kernel_spmd

F32 = mybir.dt.float32
F32R = mybir.dt.float32r
BF16 = mybir.dt.bfloat16
AF = mybir.ActivationFunctionType
OP = mybir.AluOpType
P = 128
NCORES = 8

L = 4096
D = 1024


def build_graph(Ldim=L, Ddim=D):
    nc = bacc.Bacc("TRN2", target_bir_lowering=False, debug=False, num_devices=NCORES)
    ROWS = Ldim // NCORES        # query rows per core
    MB = ROWS // P               # 128-row m-chunks per core (4)
    ZB = ROWS // P               # z-shard 128-row blocks (z shard == ROWS rows)
    SW = ROWS                    # key-tile width == z-shard width
    JT = SW // P                 # 128-row subtiles per key tile
    NT = NCORES                  # one key tile per shard
    IO = Ddim // P               # contraction chunks (8)
    AO = Ddim // P               # d_attn 128-blocks (8)
    VH = Ddim // 512             # 512-wide value column halves (2)
    HLF = ROWS // 256            # 256-row halves for PV psum pressure (2)
    scale = 1.0 / math.sqrt(Ddim)

    x_ext = nc.declare_dram_parameter("x", [ROWS, Ddim], F32, isOutput=False)
    z_ext = nc.declare_dram_parameter("z", [ROWS, Ddim], F32, isOutput=False)
    wq_ext = nc.declare_dram_parameter("Wq", [Ddim, Ddim], F32, isOutput=False)
    wk_ext = nc.declare_dram_parameter("Wk", [Ddim, Ddim], F32, isOutput=False)
    wv_ext = nc.declare_dram_parameter("Wv", [Ddim, Ddim], F32, isOutput=False)
    bq_ext = nc.declare_dram_parameter("bq", [Ddim], F32, isOutput=False)
    bk_ext = nc.declare_dram_parameter("bk", [Ddim], F32, isOutput=False)
    bv_ext = nc.declare_dram_parameter("bv", [Ddim], F32, isOutput=False)
    row0_ext = nc.declare_dram_parameter("row0", [1], F32, isOutput=False)
    out_ext = nc.declare_dram_parameter("out", [ROWS, Ddim], F32, isOutput=True)

    ident_d = nc.inline_tensor(np.eye(P, dtype=np.float32), name="ident_c")
    ones_d = nc.inline_tensor(np.ones((P, 8), np.float32), name="ones_c")
    # iota[p, j, m] = m - p - 128j ; mask keeps where iota + row0 - SW*t <= 0
    iota_np = (np.arange(ROWS)[None, None, :] - np.arange(P)[:, None, None]
               - 128 * np.arange(JT)[None, :, None]).astype(np.float32)
    iota_d = nc.inline_tensor(np.ascontiguousarray(iota_np), name="iota_c")
    # n512[p, t] = -SW*t
    n512_d = nc.inline_tensor(
        np.broadcast_to((-float(SW) * np.arange(NT, dtype=np.float32))[None, :], (P, NT)).copy(),
        name="n512_c")

    with tile.TileContext(nc) as tc:
        with tc.tile_pool(name="const", bufs=1) as constp, \
             tc.tile_pool(name="persist", bufs=1) as persist, \
             tc.tile_pool(name="dram", bufs=1, space="DRAM") as dram:
            ident = constp.tile([P, P], F32R)
            nc.gpsimd.dma_start(out=ident[:], in_=ident_d.ap())
            ones8 = constp.tile([P, 8], BF16)
            nc.gpsimd.dma_start(out=ones8[:], in_=ones_d.ap())
            iota = constp.tile([P, JT, ROWS], F32)
            nc.sync.dma_start(out=iota[:], in_=iota_d.ap())
            bvb = constp.tile([P, Ddim], F32)
            nc.gpsimd.dma_start(out=bvb[:], in_=bv_ext[:].partition_broadcast(P))
            bqs = constp.tile([P, AO], F32)
            nc.sync.dma_start(out=bqs[:], in_=bq_ext[:].rearrange("(ao p) -> p ao", p=P))
            bks = constp.tile([P, AO], F32)
            nc.sync.dma_start(out=bks[:], in_=bk_ext[:].rearrange("(ao p) -> p ao", p=P))
            row0b = constp.tile([P, 1], F32)
            nc.sync.dma_start(out=row0b[:], in_=row0_ext[:].partition_broadcast(P))
            # r0t[p, t] = row0 - 512t
            n512 = constp.tile([P, NT], F32)
            nc.sync.dma_start(out=n512[:], in_=n512_d.ap())
            r0t = constp.tile([P, NT], F32)
            nc.vector.tensor_scalar(r0t[:], n512[:], row0b[:], None, OP.add)

            QT = persist.tile([P, IO, ROWS], F32R)
            kt_b = dram.tile([Ddim, ROWS], F32R)
            v_b = dram.tile([ROWS, Ddim], BF16)
            kt_g = dram.tile([NCORES, Ddim, ROWS], F32R)
            v_g = dram.tile([NCORES, ROWS, Ddim], BF16)

            # ---------------- Phase 1: K/V projection of own z shard ----------------
            with tc.tile_pool(name="wkv", bufs=1) as wp, \
                 tc.tile_pool(name="zp", bufs=1) as zp, \
                 tc.tile_pool(name="kvs", bufs=1) as kvsp, \
                 tc.tile_pool(name="tpp", bufs=2, space="PSUM") as tpp, \
                 tc.tile_pool(name="pp", bufs=2, space="PSUM") as pp:
                wk = wp.tile([P, IO, Ddim], F32R)
                nc.gpsimd.dma_start(out=wk[:], in_=wk_ext[:].rearrange("(io p) a -> p io a", p=P))
                wv = wp.tile([P, IO, Ddim], F32R)
                nc.gpsimd.dma_start(out=wv[:], in_=wv_ext[:].rearrange("(io p) a -> p io a", p=P))
                zsb = zp.tile([P, ZB, Ddim], F32R)
                nc.gpsimd.dma_start(out=zsb[:], in_=z_ext[:].rearrange("(nb p) i -> p nb i", p=P))
                zT = zp.tile([P, IO, ROWS], F32R)
                for nb in range(ZB):
                    for io in range(IO):
                        tp = tpp.tile([P, P], F32R, tag="tp")
                        nc.tensor.transpose(tp[:], zsb[:, nb, io * P:(io + 1) * P], ident[:])
                        nc.vector.tensor_copy(zT[:, io, nb * P:(nb + 1) * P], tp[:])

                KTs = kvsp.tile([P, AO, ROWS], F32R)
                for ao in range(AO):
                    kp = pp.tile([P, ROWS], F32, tag="kp")
                    for io in range(IO):
                        nc.tensor.matmul(kp[:], wk[:, io, ao * P:(ao + 1) * P], zT[:, io, :],
                                         start=(io == 0), stop=(io == IO - 1))
                    nc.vector.tensor_scalar(KTs[:, ao, :], kp[:], bks[:, ao:ao + 1], None, OP.add)

                Vs = kvsp.tile([P, ZB, Ddim], BF16)
                for nb in range(ZB):
                    vp = pp.tile([P, Ddim], F32, tag="vp")
                    for io in range(IO):
                        for vh in range(VH):
                            nc.tensor.matmul(vp[:, vh * 512:(vh + 1) * 512],
                                             zT[:, io, nb * P:(nb + 1) * P],
                                             wv[:, io, vh * 512:(vh + 1) * 512],
                                             start=(io == 0), stop=(io == IO - 1))
                    nc.vector.tensor_tensor(Vs[:, nb, :], vp[:], bvb[:], OP.add)

                nc.sync.dma_start(out=kt_b[:].rearrange("(ao p) n -> p ao n", p=P), in_=KTs[:])
                nc.sync.dma_start(out=v_b[:].rearrange("(nb p) v -> p nb v", p=P), in_=Vs[:])

            nc.gpsimd.collective_compute(
                "AllGather", OP.bypass, replica_groups=[list(range(NCORES))],
                ins=[kt_b[:].opt()], outs=[kt_g[:].opt()])
            nc.gpsimd.collective_compute(
                "AllGather", OP.bypass, replica_groups=[list(range(NCORES))],
                ins=[v_b[:].opt()], outs=[v_g[:].opt()])

            # ---------------- Phase 2: Q^T projection (overlaps the AllGather) ----------------
            with tc.tile_pool(name="wqp", bufs=1) as wqp, \
                 tc.tile_pool(name="xp", bufs=1) as xp, \
                 tc.tile_pool(name="tpq", bufs=2, space="PSUM") as tpq, \
                 tc.tile_pool(name="qpp", bufs=2, space="PSUM") as qpp:
                wq = wqp.tile([P, IO, Ddim], F32R)
                nc.gpsimd.dma_start(out=wq[:], in_=wq_ext[:].rearrange("(io p) a -> p io a", p=P))
                xsb = xp.tile([P, MB, Ddim], F32R)
                nc.gpsimd.dma_start(out=xsb[:], in_=x_ext[:].rearrange("(mb p) i -> p mb i", p=P))
                xT = xp.tile([P, IO, ROWS], F32R)
                for mb in range(MB):
                    for io in range(IO):
                        tq = tpq.tile([P, P], F32R, tag="tq")
                        nc.tensor.transpose(tq[:], xsb[:, mb, io * P:(io + 1) * P], ident[:])
                        nc.vector.tensor_copy(xT[:, io, mb * P:(mb + 1) * P], tq[:])
                for ao in range(AO):
                    qp = qpp.tile([P, ROWS], F32, tag="qp")
                    for io in range(IO):
                        nc.tensor.matmul(qp[:], wq[:, io, ao * P:(ao + 1) * P], xT[:, io, :],
                                         start=(io == 0), stop=(io == IO - 1))
                    # fold the softmax 1/sqrt(D) into Q^T
                    nc.vector.tensor_scalar(QT[:, ao, :], qp[:], bqs[:, ao:ao + 1], float(scale),
                                            OP.add, OP.mult)

            # ---------------- Phase 3: attention over gathered K^T / V ----------------
            acc = persist.tile([P, MB, Ddim], F32)       # PV accumulator (SBUF)
            lacc = persist.tile([P, MB, 8], F32)         # row-sum accumulator
            with tc.tile_pool(name="kvt", bufs=2) as kvt, \
                 tc.tile_pool(name="esp", bufs=2) as esp, \
                 tc.tile_pool(name="mkp", bufs=2) as mkp, \
                 tc.tile_pool(name="spp", bufs=2, space="PSUM") as spp, \
                 tc.tile_pool(name="pvp", bufs=1, space="PSUM") as pvp, \
                 tc.tile_pool(name="lpp", bufs=1, space="PSUM") as lpp:
                for t in range(NT):
                    ktt = kvt.tile([P, IO, SW], F32R, tag="ktt")
                    nc.sync.dma_start(out=ktt[:], in_=kt_g[t].rearrange("(ao p) n -> p ao n", p=P))
                    vtt = kvt.tile([P, JT, Ddim], BF16, tag="vtt")
                    nc.sync.dma_start(out=vtt[:], in_=v_g[t].rearrange("(nb p) v -> p nb v", p=P))

                    es = esp.tile([P, JT, ROWS], BF16, tag="es")
                    for j in range(JT):
                        sp = spp.tile([P, ROWS], F32, tag="sp")
                        for io in range(IO):
                            nc.tensor.matmul(sp[:], ktt[:, io, j * P:(j + 1) * P],
                                             QT[:, io, :], start=(io == 0), stop=(io == IO - 1))
                        nc.scalar.activation(es[:, j, :], sp[:], AF.Exp)
                    # multiplicative causal mask: keep where iota + (row0 - 512t) <= 0
                    mk = mkp.tile([P, JT, ROWS], BF16, tag="mk")
                    nc.vector.tensor_scalar(mk[:], iota[:], r0t[:, t:t + 1], 0.0,
                                            OP.add, OP.is_le)
                    nc.vector.tensor_tensor(es[:], es[:], mk[:], OP.mult)

                    for h in range(HLF):
                        pvs = [pvp.tile([P, Ddim], F32, tag=f"pv{mc}", name=f"pv{mc}_{t}_{h}") for mc in range(2)]
                        lps = [lpp.tile([P, 8], F32, tag=f"lp{mc}", name=f"lp{mc}_{t}_{h}") for mc in range(2)]
                        for j in range(JT):
                            for mc in range(2):
                                m0 = h * 256 + mc * P
                                lhs = es[:, j, m0:m0 + P]
                                nc.tensor.matmul(lps[mc][:], lhs, ones8[:],
                                                 start=(j == 0), stop=(j == JT - 1))
                                for vh in range(VH):
                                    nc.tensor.matmul(pvs[mc][:, vh * 512:(vh + 1) * 512],
                                                     lhs, vtt[:, j, vh * 512:(vh + 1) * 512],
                                                     start=(j == 0), stop=(j == JT - 1))
                        for mc in range(2):
                            gmc = 2 * h + mc
                            if t == 0:
                                nc.vector.tensor_copy(acc[:, gmc, :], pvs[mc][:])
                                nc.vector.tensor_copy(lacc[:, gmc, :], lps[mc][:])
                            else:
                                nc.vector.tensor_tensor(acc[:, gmc, :], acc[:, gmc, :],
                                                        pvs[mc][:], OP.add)
                                nc.vector.tensor_tensor(lacc[:, gmc, :], lacc[:, gmc, :],
                                                        lps[mc][:], OP.add)

                # normalize and write out
                with tc.tile_pool(name="recp", bufs=1) as recp:
                    for gmc in range(MB):
                        rec = recp.tile([P, 1], F32, tag="rec")
                        nc.vector.reciprocal(rec[:], lacc[:, gmc, 0:1])
                        nc.vector.tensor_scalar(acc[:, gmc, :], acc[:, gmc, :], rec[:],
                                                None, OP.mult)
                    nc.sync.dma_start(out=out_ext[:].rearrange("(mb p) v -> p mb v", p=P),
                                      in_=acc[:])
    nc.compile()
    return nc


_GRAPH_CACHE = {}


def _get_graph(Ldim=L, Ddim=D):
    key = (Ldim, Ddim)
    if key not in _GRAPH_CACHE:
        _GRAPH_CACHE[key] = build_graph(Ldim, Ddim)
    return _GRAPH_CACHE[key]


def kernel(x, z, Wq, bq, Wk, bk, Wv, bv):
    x = np.ascontiguousarray(np.asarray(x, dtype=np.float32))
    z = np.ascontiguousarray(np.asarray(z, dtype=np.float32))
    Ldim, Ddim = x.shape
    nc = _get_graph(Ldim, Ddim)
    ROWS = Ldim // NCORES
    common = {
        "Wq": np.asarray(Wq, np.float32), "bq": np.asarray(bq, np.float32),
        "Wk": np.asarray(Wk, np.float32), "bk": np.asarray(bk, np.float32),
        "Wv": np.asarray(Wv, np.float32), "bv": np.asarray(bv, np.float32),
    }
    in_maps = []
    for c in range(NCORES):
        m = dict(common)
        m["x"] = x[ROWS * c:ROWS * (c + 1)]
        m["z"] = z[ROWS * c:ROWS * (c + 1)]
        m["row0"] = np.array([ROWS * c], dtype=np.float32)
        in_maps.append(m)
    res = run_bass_kernel_spmd(nc, in_maps, core_ids=list(range(NCORES)))
    out = np.empty((Ldim, Ddim), dtype=np.float32)
    for c in range(NCORES):
        out[ROWS * c:ROWS * (c + 1)] = res.results[c]["out"]
    return out


# revision 7
# speedup vs baseline: 1.1639x; 1.1423x over previous
"""Distributed Trainium2 Bass kernel: masked (upper-triangular) attention.

reference (L=4096, D=1024, fp32):
    Q = x @ Wq + bq ; K = z @ Wk + bk ; V = z @ Wv + bv
    S = Q @ K.T ; S[row > col] = -inf
    out = softmax(S / sqrt(D)) @ V

Strategy (8 NeuronCores, one TRN2 chip, SPMD):
  - Sequence parallel on query rows: core c owns rows [512c, 512c+512).
  - K/V projection sharded over z rows (512/core), AllGathered in bf16
    (K stored transposed [D, L] blocked by shard, V natural [L, D]).
  - Attention computed as S^T tiles (keys on partitions) so the P^T needed by
    the PV matmul comes straight out of the softmax with no transposes.
  - Softmax without max-subtraction (scores here are O(1), exp can't overflow
    in fp32); mask applied multiplicatively after exp, built at runtime from
    an iota constant + a per-core row0 scalar input, keeping one graph valid
    for all cores (SPMD - no per-core control flow).
  - Matmuls in bf16 with fp32 PSUM accumulation (end-to-end rel err ~3e-3).
"""

import math

import numpy as np

import concourse.mybir as mybir
import concourse.tile as tile
from concourse import bacc
from concourse.bass_utils import run_bass_kernel_spmd

F32 = mybir.dt.float32
BF16 = mybir.dt.bfloat16
AF = mybir.ActivationFunctionType
OP = mybir.AluOpType
P = 128
NCORES = 8

L = 4096
D = 1024


def build_graph(Ldim=L, Ddim=D):
    nc = bacc.Bacc("TRN2", target_bir_lowering=False, debug=False, num_devices=NCORES)
    ROWS = Ldim // NCORES        # query rows per core
    MB = ROWS // P               # 128-row m-chunks per core (4)
    ZB = ROWS // P               # z-shard 128-row blocks (4)
    SW = ROWS                    # key-tile width == z-shard width (512)
    JT = SW // P                 # 128-row subtiles per key tile (4)
    NT = NCORES                  # one key tile per shard
    IO = Ddim // P               # contraction chunks (8)
    AO = Ddim // P               # d_attn 128-blocks (8)
    VH = Ddim // 512             # 512-wide value column halves (2)
    HLF = ROWS // 256            # 256-row halves for PV psum pressure (2)
    scale = 1.0 / math.sqrt(Ddim)

    x_ext = nc.declare_dram_parameter("x", [ROWS, Ddim], F32, isOutput=False)
    z_ext = nc.declare_dram_parameter("z", [ROWS, Ddim], F32, isOutput=False)
    wq_ext = nc.declare_dram_parameter("Wq", [Ddim, Ddim], F32, isOutput=False)
    wk_ext = nc.declare_dram_parameter("Wk", [Ddim, Ddim], F32, isOutput=False)
    wv_ext = nc.declare_dram_parameter("Wv", [Ddim, Ddim], F32, isOutput=False)
    bq_ext = nc.declare_dram_parameter("bq", [Ddim], F32, isOutput=False)
    bk_ext = nc.declare_dram_parameter("bk", [Ddim], F32, isOutput=False)
    bv_ext = nc.declare_dram_parameter("bv", [Ddim], F32, isOutput=False)
    row0_ext = nc.declare_dram_parameter("row0", [1], F32, isOutput=False)
    out_ext = nc.declare_dram_parameter("out", [ROWS, Ddim], F32, isOutput=True)

    ident_d = nc.inline_tensor(np.eye(P, dtype=np.float32), name="ident_c")
    ones_d = nc.inline_tensor(np.ones((P, 8), np.float32), name="ones_c")
    # iota[p, j, m] = m - p - 128j ; mask keeps where iota + row0 - SW*t <= 0
    iota_np = (np.arange(ROWS)[None, None, :] - np.arange(P)[:, None, None]
               - 128 * np.arange(JT)[None, :, None]).astype(np.float32)
    iota_d = nc.inline_tensor(np.ascontiguousarray(iota_np), name="iota_c")
    nSWt_d = nc.inline_tensor(
        np.broadcast_to((-float(SW) * np.arange(NT, dtype=np.float32))[None, :], (P, NT)).copy(),
        name="nswt_c")

    with tile.TileContext(nc) as tc:
        with tc.tile_pool(name="const", bufs=1) as constp, \
             tc.tile_pool(name="persist", bufs=1) as persist, \
             tc.tile_pool(name="dram", bufs=1, space="DRAM") as dram:
            ident = constp.tile([P, P], F32)
            nc.sync.dma_start(out=ident[:], in_=ident_d.ap())
            ones_f = constp.tile([P, 8], F32)
            nc.sync.dma_start(out=ones_f[:], in_=ones_d.ap())
            ones8 = constp.tile([P, 8], BF16)
            nc.vector.tensor_copy(ones8[:], ones_f[:])
            iota = constp.tile([P, JT, ROWS], F32)
            nc.sync.dma_start(out=iota[:], in_=iota_d.ap())
            bvb = constp.tile([P, Ddim], F32)
            nc.sync.dma_start(out=bvb[:], in_=bv_ext[:].partition_broadcast(P))
            bqs = constp.tile([P, AO], F32)
            nc.sync.dma_start(out=bqs[:], in_=bq_ext[:].rearrange("(ao p) -> p ao", p=P))
            bks = constp.tile([P, AO], F32)
            nc.sync.dma_start(out=bks[:], in_=bk_ext[:].rearrange("(ao p) -> p ao", p=P))
            row0b = constp.tile([P, 1], F32)
            nc.sync.dma_start(out=row0b[:], in_=row0_ext[:].partition_broadcast(P))
            nswt = constp.tile([P, NT], F32)
            nc.sync.dma_start(out=nswt[:], in_=nSWt_d.ap())
            r0t = constp.tile([P, NT], F32)
            nc.vector.tensor_scalar(r0t[:], nswt[:], row0b[:], None, OP.add)

            QT = persist.tile([P, IO, ROWS], BF16)
            kt_b = dram.tile([Ddim, ROWS], BF16)
            v_b = dram.tile([ROWS, Ddim], BF16)
            kt_g = dram.tile([NCORES, Ddim, ROWS], BF16)
            v_g = dram.tile([NCORES, ROWS, Ddim], BF16)

            # ---------------- Phase 1: K/V projection of own z shard ----------------
            with tc.tile_pool(name="wst", bufs=3) as wst, \
                 tc.tile_pool(name="wkv", bufs=1) as wp, \
                 tc.tile_pool(name="zp", bufs=1) as zp, \
                 tc.tile_pool(name="kvs", bufs=1) as kvsp, \
                 tc.tile_pool(name="tpp", bufs=2, space="PSUM") as tpp, \
                 tc.tile_pool(name="pp", bufs=2, space="PSUM") as pp:
                wk = wp.tile([P, IO, Ddim], BF16)
                wv = wp.tile([P, IO, Ddim], BF16)
                for wi, (wtile, wext) in enumerate(((wk, wk_ext), (wv, wv_ext))):
                    for io in range(IO):
                        ws = wst.tile([P, Ddim], F32, tag="ws", name=f"ws_{wi}_{io}")
                        nc.sync.dma_start(out=ws[:], in_=wext[io * P:(io + 1) * P, :])
                        nc.vector.tensor_copy(wtile[:, io, :], ws[:])
                zsb = zp.tile([P, ZB, Ddim], F32)
                nc.sync.dma_start(out=zsb[:], in_=z_ext[:].rearrange("(nb p) i -> p nb i", p=P))
                zT = zp.tile([P, IO, ROWS], BF16)
                for nb in range(ZB):
                    for io in range(IO):
                        tp = tpp.tile([P, P], F32, tag="tp", name=f"tp_{nb}_{io}")
                        nc.tensor.transpose(tp[:], zsb[:, nb, io * P:(io + 1) * P], ident[:])
                        nc.vector.tensor_copy(zT[:, io, nb * P:(nb + 1) * P], tp[:])

                KTs = kvsp.tile([P, AO, ROWS], BF16)
                for ao in range(AO):
                    kp = pp.tile([P, ROWS], F32, tag="kp", name=f"kp_{ao}")
                    for io in range(IO):
                        nc.tensor.matmul(kp[:], wk[:, io, ao * P:(ao + 1) * P], zT[:, io, :],
                                         start=(io == 0), stop=(io == IO - 1))
                    nc.vector.tensor_scalar(KTs[:, ao, :], kp[:], bks[:, ao:ao + 1], None, OP.add)
                nc.sync.dma_start(out=kt_b[:].rearrange("(ao p) n -> p ao n", p=P), in_=KTs[:])

                Vs = kvsp.tile([P, ZB, Ddim], BF16)
                for nb in range(ZB):
                    vp = pp.tile([P, Ddim], F32, tag="vp", name=f"vp_{nb}")
                    for io in range(IO):
                        for vh in range(VH):
                            nc.tensor.matmul(vp[:, vh * 512:(vh + 1) * 512],
                                             zT[:, io, nb * P:(nb + 1) * P],
                                             wv[:, io, vh * 512:(vh + 1) * 512],
                                             start=(io == 0), stop=(io == IO - 1))
                    nc.vector.tensor_tensor(Vs[:, nb, :], vp[:], bvb[:], OP.add)
                nc.sync.dma_start(out=v_b[:].rearrange("(nb p) v -> p nb v", p=P), in_=Vs[:])

            nc.gpsimd.collective_compute(
                "AllGather", OP.bypass, replica_groups=[list(range(NCORES))],
                ins=[kt_b[:].opt()], outs=[kt_g[:].opt()])
            nc.gpsimd.collective_compute(
                "AllGather", OP.bypass, replica_groups=[list(range(NCORES))],
                ins=[v_b[:].opt()], outs=[v_g[:].opt()])

            # ---------------- Phase 2: Q^T projection (overlaps the AllGather) ----------------
            with tc.tile_pool(name="wqst", bufs=3) as wqst, \
                 tc.tile_pool(name="wqp", bufs=1) as wqp, \
                 tc.tile_pool(name="xp", bufs=1) as xp, \
                 tc.tile_pool(name="tpq", bufs=2, space="PSUM") as tpq, \
                 tc.tile_pool(name="qpp", bufs=2, space="PSUM") as qpp:
                wq = wqp.tile([P, IO, Ddim], BF16)
                for io in range(IO):
                    wqs = wqst.tile([P, Ddim], F32, tag="wqs", name=f"wqs_{io}")
                    nc.sync.dma_start(out=wqs[:], in_=wq_ext[io * P:(io + 1) * P, :])
                    nc.vector.tensor_copy(wq[:, io, :], wqs[:])
                xsb = xp.tile([P, MB, Ddim], F32)
                nc.sync.dma_start(out=xsb[:], in_=x_ext[:].rearrange("(mb p) i -> p mb i", p=P))
                xT = xp.tile([P, IO, ROWS], BF16)
                for mb in range(MB):
                    for io in range(IO):
                        tq = tpq.tile([P, P], F32, tag="tq", name=f"tq_{mb}_{io}")
                        nc.tensor.transpose(tq[:], xsb[:, mb, io * P:(io + 1) * P], ident[:])
                        nc.vector.tensor_copy(xT[:, io, mb * P:(mb + 1) * P], tq[:])
                for ao in range(AO):
                    qp = qpp.tile([P, ROWS], F32, tag="qp", name=f"qp_{ao}")
                    for io in range(IO):
                        nc.tensor.matmul(qp[:], wq[:, io, ao * P:(ao + 1) * P], xT[:, io, :],
                                         start=(io == 0), stop=(io == IO - 1))
                    # fold the softmax 1/sqrt(D) into Q^T
                    nc.vector.tensor_scalar(QT[:, ao, :], qp[:], bqs[:, ao:ao + 1], float(scale),
                                            OP.add, OP.mult)

            # ---------------- Phase 3: attention over gathered K^T / V ----------------
            acc = persist.tile([P, MB, Ddim], F32)       # PV accumulator (SBUF)
            lacc = persist.tile([P, MB, 8], F32)         # row-sum accumulator
            with tc.tile_pool(name="kvt", bufs=2) as kvt, \
                 tc.tile_pool(name="esp", bufs=2) as esp, \
                 tc.tile_pool(name="mkp", bufs=2) as mkp, \
                 tc.tile_pool(name="spp", bufs=2, space="PSUM") as spp, \
                 tc.tile_pool(name="pvp", bufs=1, space="PSUM") as pvp, \
                 tc.tile_pool(name="lpp", bufs=1, space="PSUM") as lpp:
                for t in range(NT):
                    ktt = kvt.tile([P, IO, SW], BF16, tag="ktt", name=f"ktt_{t}")
                    nc.sync.dma_start(out=ktt[:], in_=kt_g[t].rearrange("(ao p) n -> p ao n", p=P))
                    vtt = kvt.tile([P, JT, Ddim], BF16, tag="vtt", name=f"vtt_{t}")
                    nc.sync.dma_start(out=vtt[:], in_=v_g[t].rearrange("(nb p) v -> p nb v", p=P))

                    es = esp.tile([P, JT, ROWS], BF16, tag="es", name=f"es_{t}")
                    for j in range(JT):
                        sp = spp.tile([P, ROWS], F32, tag="sp", name=f"sp_{t}_{j}")
                        for io in range(IO):
                            nc.tensor.matmul(sp[:], ktt[:, io, j * P:(j + 1) * P],
                                             QT[:, io, :], start=(io == 0), stop=(io == IO - 1))
                        nc.scalar.activation(es[:, j, :], sp[:], AF.Exp)
                    # multiplicative causal mask: keep where iota + (row0 - SW*t) <= 0
                    mk = mkp.tile([P, JT, ROWS], BF16, tag="mk", name=f"mk_{t}")
                    nc.vector.tensor_scalar(mk[:], iota[:], r0t[:, t:t + 1], 0.0,
                                            OP.add, OP.is_le)
                    nc.vector.tensor_tensor(es[:], es[:], mk[:], OP.mult)

                    for h in range(HLF):
                        pvs = [pvp.tile([P, Ddim], F32, tag=f"pv{mc}", name=f"pv{mc}_{t}_{h}")
                               for mc in range(2)]
                        lps = [lpp.tile([P, 8], F32, tag=f"lp{mc}", name=f"lp{mc}_{t}_{h}")
                               for mc in range(2)]
                        for j in range(JT):
                            for mc in range(2):
                                m0 = h * 256 + mc * P
                                lhs = es[:, j, m0:m0 + P]
                                nc.tensor.matmul(lps[mc][:], lhs, ones8[:],
                                                 start=(j == 0), stop=(j == JT - 1))
                                for vh in range(VH):
                                    nc.tensor.matmul(pvs[mc][:, vh * 512:(vh + 1) * 512],
                                                     lhs, vtt[:, j, vh * 512:(vh + 1) * 512],
                                                     start=(j == 0), stop=(j == JT - 1))
                        for mc in range(2):
                            gmc = 2 * h + mc
                            if t == 0:
                                nc.vector.tensor_copy(acc[:, gmc, :], pvs[mc][:])
                                nc.vector.tensor_copy(lacc[:, gmc, :], lps[mc][:])
                            else:
                                nc.vector.tensor_tensor(acc[:, gmc, :], acc[:, gmc, :],
                                                        pvs[mc][:], OP.add)
                                nc.vector.tensor_tensor(lacc[:, gmc, :], lacc[:, gmc, :],
                                                        lps[mc][:], OP.add)

                # normalize and write out
                with tc.tile_pool(name="recp", bufs=1) as recp:
                    for gmc in range(MB):
                        rec = recp.tile([P, 1], F32, tag="rec", name=f"rec_{gmc}")
                        nc.vector.reciprocal(rec[:], lacc[:, gmc, 0:1])
                        nc.vector.tensor_scalar(acc[:, gmc, :], acc[:, gmc, :], rec[:],
                                                None, OP.mult)
                    nc.sync.dma_start(out=out_ext[:].rearrange("(mb p) v -> p mb v", p=P),
                                      in_=acc[:])
    nc.compile()
    return nc


_GRAPH_CACHE = {}


def _get_graph(Ldim=L, Ddim=D):
    key = (Ldim, Ddim)
    if key not in _GRAPH_CACHE:
        _GRAPH_CACHE[key] = build_graph(Ldim, Ddim)
    return _GRAPH_CACHE[key]


def kernel(x, z, Wq, bq, Wk, bk, Wv, bv):
    x = np.ascontiguousarray(np.asarray(x, dtype=np.float32))
    z = np.ascontiguousarray(np.asarray(z, dtype=np.float32))
    Ldim, Ddim = x.shape
    nc = _get_graph(Ldim, Ddim)
    ROWS = Ldim // NCORES
    common = {
        "Wq": np.ascontiguousarray(np.asarray(Wq, np.float32)),
        "bq": np.ascontiguousarray(np.asarray(bq, np.float32)),
        "Wk": np.ascontiguousarray(np.asarray(Wk, np.float32)),
        "bk": np.ascontiguousarray(np.asarray(bk, np.float32)),
        "Wv": np.ascontiguousarray(np.asarray(Wv, np.float32)),
        "bv": np.ascontiguousarray(np.asarray(bv, np.float32)),
    }
    in_maps = []
    for c in range(NCORES):
        m = dict(common)
        m["x"] = x[ROWS * c:ROWS * (c + 1)]
        m["z"] = z[ROWS * c:ROWS * (c + 1)]
        m["row0"] = np.array([ROWS * c], dtype=np.float32)
        in_maps.append(m)
    res = run_bass_kernel_spmd(nc, in_maps, core_ids=list(range(NCORES)))
    out = np.empty((Ldim, Ddim), dtype=np.float32)
    for c in range(NCORES):
        out[ROWS * c:ROWS * (c + 1)] = res.results[c]["out"]
    return out
